# revision 1
# baseline (speedup 1.0000x reference)
"""BCQConv1D TRN2 kernel: out[b,s,o] = x[b,s,:] @ W[o,:]^T + bias[o],
W[o, g*A+a] = sum_qb alpha[o,g,qb] * binary[o,g,a,qb].

Sharding: column-parallel — alpha/binary/bias split along out_features
across the 8 NeuronCores, x replicated (each core computes the full
batch for its 512 output features).

Per core: reconstruct the W^T shard on device (DVE fused mul-add over
the 3 BCQ bit planes + PE transpose), keep it resident in SBUF as
float32r, then stream x^T tiles and run N=512 float32r matmuls (full
bf16 rate on the PE, ~13 effective mantissa bits) accumulating over
K=4096 in PSUM. Bias is broadcast once via a K=1 ones-matmul and folded
into the PSUM->SBUF output add on the DVE.

Host side only slices/relayouts inputs (x is passed transposed/tiled
[128, KT, BS] so every DMA line is 2KB contiguous).
"""

import numpy as np

import concourse.bass as bass
import concourse.tile as tile
from concourse import bacc, mybir
from concourse.bass_utils import run_bass_kernel_spmd
from concourse.masks import make_identity

# Problem shape (hardcoded per contest contract)
B, S, I, O = 4, 2048, 4096, 4096
G, A, QB = 32, 128, 3
BS = B * S  # 8192
P = 128
KT = I // P  # 32 k-tiles (== groups: i = g*A + a, A == P)

# Sharding
N_CORES = 8
O_WAYS = 8
BS_WAYS = 1
O_SH = O // O_WAYS  # per-core out features
BS_SH = BS // BS_WAYS  # per-core batch rows
NFREE = 512  # matmul moving free dim (one PSUM bank of fp32)
NB = O_SH // NFREE  # o-blocks per core
BCHUNK = 512  # bs columns fetched per x DMA
NSUB = BCHUNK // P  # matmul chains per x chunk
GMERGE = 2  # binary groups per recon DMA

F32 = mybir.dt.float32
F32R = mybir.dt.float32r


def build_nc():
    nc = bacc.Bacc(target_bir_lowering=False)
    xt_d = nc.declare_dram_parameter("xt", [P, KT, BS_SH], F32R, isOutput=False)
    alpha_d = nc.declare_dram_parameter("alpha", [O_SH, G, QB], F32, isOutput=False)
    binary_d = nc.declare_dram_parameter("binary", [O_SH, G, A, QB], F32, isOutput=False)
    bias_d = nc.declare_dram_parameter("bias", [O_SH], F32, isOutput=False)
    out_d = nc.declare_dram_parameter("out", [BS_SH, O_SH], F32, isOutput=True)

    OT = O_SH // P  # o-tiles for recon
    add = mybir.AluOpType.add
    mult = mybir.AluOpType.mult

    with tile.TileContext(nc) as tc:
        with (
            tc.tile_pool(name="const", bufs=1) as cpool,
            tc.tile_pool(name="wt", bufs=1) as wtpool,
            tc.tile_pool(name="rec", bufs=4) as rec,
            tc.tile_pool(name="wog", bufs=4) as wog_pool,
            tc.tile_pool(name="xp", bufs=10) as xp,
            tc.tile_pool(name="op", bufs=8) as op,
            tc.tile_pool(name="psum", bufs=8, space="PSUM") as pp,
        ):
            # --- constants ---
            ident = cpool.tile([P, P], F32, name="ident")
            make_identity(nc, ident)
            ones = cpool.tile([1, P], F32, name="ones")
            nc.vector.memset(ones, 1.0)
            bias_row = cpool.tile([1, O_SH], F32, name="bias_row")
            nc.sync.dma_start(out=bias_row, in_=bias_d.ap().unsqueeze(0))
            bias_bc = cpool.tile([P, O_SH], F32, name="bias_bc")
            for j in range(NB):
                pbt = pp.tile([P, NFREE], F32, tag="ps", name=f"psb{j}")
                nc.tensor.matmul(
                    pbt, ones, bias_row[:, j * NFREE : (j + 1) * NFREE],
                    start=True, stop=True,
                )
                nc.vector.tensor_copy(
                    out=bias_bc[:, j * NFREE : (j + 1) * NFREE], in_=pbt
                )

            # --- alpha (per-partition scalars), all o-tiles resident ---
            alpha_sb = []
            for ot in range(OT):
                at = cpool.tile([P, G, QB], F32, name=f"alpha{ot}")
                nc.sync.dma_start(out=at, in_=alpha_d.ap()[ot * P : (ot + 1) * P])
                alpha_sb.append(at)

            # --- W^T shard, resident, one tile per k-tile (== group) ---
            wt_tiles = [
                wtpool.tile([P, O_SH], F32R, tag=f"wt{k}", name=f"wt{k}")
                for k in range(KT)
            ]

            # --- reconstruction: W[o, g*A + a] then PE-transpose to W^T ---
            for gp in range(G // GMERGE):
                for ot in range(OT):
                    bt = rec.tile([P, GMERGE, A, QB], F32, tag="bt")
                    nc.sync.dma_start(
                        out=bt,
                        in_=binary_d.ap()[
                            ot * P : (ot + 1) * P,
                            gp * GMERGE : (gp + 1) * GMERGE,
                        ],
                    )
                    at = alpha_sb[ot]
                    for gg in range(GMERGE):
                        g = gp * GMERGE + gg
                        w_og = wog_pool.tile([P, P], F32, tag="wog")
                        nc.vector.tensor_scalar_mul(
                            w_og, bt[:, gg, :, 0], at[:, g, 0:1]
                        )
                        nc.vector.scalar_tensor_tensor(
                            w_og, bt[:, gg, :, 1], at[:, g, 1:2], w_og, mult, add
                        )
                        nc.vector.scalar_tensor_tensor(
                            w_og, bt[:, gg, :, 2], at[:, g, 2:3], w_og, mult, add
                        )
                        ptt = pp.tile([P, P], F32, tag="ps", name=f"ptr{g}_{ot}")
                        nc.tensor.transpose(ptt, w_og, ident)
                        nc.vector.tensor_copy(
                            out=wt_tiles[g][:, ot * P : (ot + 1) * P], in_=ptt
                        )

            # --- main matmul: out[bs, o] = x^T.T @ W^T (+bias) ---
            n_chunks = BS_SH // BCHUNK
            for c in range(n_chunks):
                psums = [
                    [
                        pp.tile([P, NFREE], F32, tag="ps", name=f"mm{c}_{s}_{j}")
                        for j in range(NB)
                    ]
                    for s in range(NSUB)
                ]
                for k in range(KT):
                    xt_t = xp.tile([P, BCHUNK], F32R, tag="xt")
                    dma_eng = nc.sync if k % 2 == 0 else nc.scalar
                    dma_eng.dma_start(
                        out=xt_t,
                        in_=xt_d.ap()[:, k, c * BCHUNK : (c + 1) * BCHUNK],
                    )
                    for s in range(NSUB):
                        for j in range(NB):
                            nc.tensor.matmul(
                                psums[s][j],
                                xt_t[:, s * P : (s + 1) * P],
                                wt_tiles[k][:, j * NFREE : (j + 1) * NFREE],
                                start=(k == 0),
                                stop=(k == KT - 1),
                            )
                for s in range(NSUB):
                    for j in range(NB):
                        os_t = op.tile([P, NFREE], F32, tag="os")
                        nc.vector.tensor_tensor(
                            out=os_t,
                            in0=psums[s][j],
                            in1=bias_bc[:, j * NFREE : (j + 1) * NFREE],
                            op=add,
                        )
                        nc.sync.dma_start(
                            out=out_d.ap()[
                                c * BCHUNK + s * P : c * BCHUNK + (s + 1) * P,
                                j * NFREE : (j + 1) * NFREE,
                            ],
                            in_=os_t,
                        )

    if not nc.is_finalized():
        nc.finalize()
    return nc


def shard_inputs(x, alpha, bias, binary):
    """Host-side slicing/relayout only. Returns per-core input maps."""
    x2 = np.ascontiguousarray(x).reshape(BS, I)
    # xtp[p, k, s] = x2[s, k*P + p]  -> every DMA line is bs-contiguous
    xtp = np.ascontiguousarray(x2.T.reshape(KT, P, BS).transpose(1, 0, 2))
    alpha = np.ascontiguousarray(alpha)
    binary = np.ascontiguousarray(binary)
    bias = np.ascontiguousarray(bias)

    in_maps = []
    for c in range(N_CORES):
        oc, bc = divmod(c, BS_WAYS)
        osl = slice(oc * O_SH, (oc + 1) * O_SH)
        if BS_WAYS == 1:
            xc = xtp
        else:
            xc = np.ascontiguousarray(xtp[:, :, bc * BS_SH : (bc + 1) * BS_SH])
        in_maps.append(
            {
                "xt": xc,
                "alpha": alpha[osl],
                "binary": binary[osl],
                "bias": bias[osl],
            }
        )
    return in_maps


def assemble_output(results):
    out = np.empty((BS, O), dtype=np.float32)
    for c in range(N_CORES):
        oc, bc = divmod(c, BS_WAYS)
        out[
            bc * BS_SH : (bc + 1) * BS_SH, oc * O_SH : (oc + 1) * O_SH
        ] = results[c]["out"]
    return out.reshape(B, S, O)


_NC_CACHE = None


def kernel(x, alpha, bias, binary):
    global _NC_CACHE
    if _NC_CACHE is None:
        _NC_CACHE = build_nc()
    nc = _NC_CACHE
    in_maps = shard_inputs(
        np.asarray(x, dtype=np.float32),
        np.asarray(alpha, dtype=np.float32),
        np.asarray(bias, dtype=np.float32),
        np.asarray(binary, dtype=np.float32),
    )
    res = run_bass_kernel_spmd(nc, in_maps, list(range(N_CORES)))
    return assemble_output(res.results)

